# revision 40
# baseline (speedup 1.0000x reference)
"""Trainium2 Bass kernel for nn_DiscretePredictor (gnn_message_passing).

Reference computation (per batch b of 256, n=32 objects, d=128):
    src/tgt = all n*n ordered pairs (i,j), e = edges[b, i*n+j] in {0,1}
    need[(b,i,j)] = [state[b,i]*e, state[b,j]*e]              (2d = 256)
    msgs = MLP1(need) = Lin(256->256) -> BN(train) -> LeakyReLU -> Lin(256->128)
    agg[b,i] = sum_j msgs[b,i,j]
    out = MLP2([state, agg]) same structure (256->256->BN->LReLU->128)

Key algebraic facts exploited:
  1. need @ W1.T = e*(u_i + v_j) with u = state@W1a.T, v = state@W1b.T
     (W1 = [W1a | W1b] split along input dim), so the 262144x256x256 matmul
     collapses to two 8192x128x256 matmuls plus an elementwise outer-add.
  2. sum_j commutes with the second linear layer:
     agg = (sum_j LeakyReLU(BN(h))) @ W2.T + n*b2.
  3. e in {0,1}  =>  LeakyReLU masking is exact: the masked pre-activation is
     H = e*(u_i+v_j); rows with e=0 reduce to the constant LeakyReLU(z0).
  4. Training-mode BN uses global batch stats => two tiny (128x4) AllReduduces
     across the 8 cores.  The BN linear bias b1 cancels:
     BN(h) = a*H + (beta - mean(H)*a), a = gamma*rsqrt(var(H)+eps).

Sharding: data-parallel over batch (32 batches per core), params replicated.

Per-core dataflow (feature-major: features on SBUF partitions).  The BN1
stats are computed WITHOUT materializing H, so the cross-core stats barrier
happens before the big elementwise phase and H never has to be spilled:
  stateT [128d, 1024(b,i)] --PE--> UT/VT [2][128f, 1024]
  sum(H)   = sum_i deg*u + sum_j cdeg*v              (tiny DVE mul+reduce)
  sum(H^2) = sum deg*u^2 + sum cdeg*v^2 + 2*w1a_f^T M w1b_f,
             M = sum_b S_b^T E_b S_b                 (small PE matmuls via
             block-diag E_b^T tiles loaded from a host-transposed edges copy)
  AllReduce [128,4] -> BN coeffs a1[f], z0[f]
  fold a1 into the weights: redo u/v matmuls with a1*W1a, a1*W1b -> UA/VA
  big phase, per group g (4 batches x 2 f-halves, 16 tiles of [128, 4096]):
    DVE  tensor_add : W = ua_i + va_j     (stride-0 broadcast APs)
    POOL tensor_mul : H = W * E_bcast     (e in {0,1}: exact masking)
    ACT  Prelu      : m = LeakyReLU(H + z0)  (z0 via per-partition bias)
    DVE  reduce_sum : msum[f,(b,i)] += sum_j m   (j innermost)
  PE: aggT = W2 @ msum + 32*b2 ; H2 = FW1 @ [stateT; aggT]
  ACT copy+accum / Square+accum -> layer-2 stats; AllReduce; Prelu -> m2
  PE: outT = FW2 @ m2 + fb2 ; DMA out (host transposes back)

Engine budget per core (cost model): DVE ~138us (add+reduce), POOL ~130us
(mask), ACT ~75us, PE ~35us; modelled end-to-end ~293us.
"""

import os
import sys

for p in ("/opt/trn_rl_repo", "/root/.axon_site", "/root/.axon_site/_ro/trn_rl_repo",
          "/root/.axon_site/_ro/pypackages"):
    if os.path.isdir(p) and p not in sys.path:
        sys.path.append(p)

import numpy as np

import concourse.bass as bass
import concourse.mybir as mybir
import concourse.tile as tile
from concourse import bacc
from concourse.bass_utils import run_bass_kernel_spmd

F32 = mybir.dt.float32
AF = mybir.ActivationFunctionType
ALU = mybir.AluOpType

B = 256          # global batch
NOBJ = 32        # objects per batch
D = 128          # object dim
F = 256          # hidden width (both MLPs)
NCORES = 8
NB = B // NCORES          # batches per core = 32
ROWS = NB * NOBJ          # (b,i) rows per core = 1024
GB = 4                    # batches per stage-B group
NG = NB // GB             # 8 groups
CG = GB * NOBJ * NOBJ     # stage-B cols per group = 4096
N1 = float(B * NOBJ * NOBJ)   # BN1 row count (global) = 262144
N2 = float(B * NOBJ)          # BN2 row count (global) = 8192
EPS = 1e-5
SLOPE = 0.01
# debug: skip cross-core allreduce (stats become shard-local; wrong numerics)
NO_CC = os.environ.get("BASS_NO_CC", "0") == "1"
STAGE = int(os.environ.get("BASS_STAGE", "9"))  # debug: emit pipeline prefix only
SUB = int(os.environ.get("BASS_SUB", "9"))      # debug: const-load subset


def _build_nc():
    nc = bacc.Bacc("TRN2", target_bir_lowering=False, debug=False,
                   enable_asserts=True, num_devices=NCORES)

    # ---- per-core device I/O ----
    stateT_d = nc.dram_tensor("stateT", [D, ROWS], F32, kind="ExternalInput")
    edges_d = nc.dram_tensor("edges_s", [NB, NOBJ * NOBJ], F32, kind="ExternalInput")
    w1aT_d = nc.dram_tensor("w1aT", [D, F], F32, kind="ExternalInput")
    w1bT_d = nc.dram_tensor("w1bT", [D, F], F32, kind="ExternalInput")
    w2T_d = nc.dram_tensor("w2T", [F, D], F32, kind="ExternalInput")
    fw1T_d = nc.dram_tensor("fw1T", [2 * D, F], F32, kind="ExternalInput")
    fw2T_d = nc.dram_tensor("fw2T", [F, D], F32, kind="ExternalInput")
    g1_d = nc.dram_tensor("g1", [F], F32, kind="ExternalInput")
    be1_d = nc.dram_tensor("be1", [F], F32, kind="ExternalInput")
    b2_d = nc.dram_tensor("b2", [D], F32, kind="ExternalInput")
    g2_d = nc.dram_tensor("g2", [F], F32, kind="ExternalInput")
    be2_d = nc.dram_tensor("be2", [F], F32, kind="ExternalInput")
    fb2_d = nc.dram_tensor("fb2", [D], F32, kind="ExternalInput")
    staterm_d = nc.dram_tensor("state_rm", [ROWS, D], F32, kind="ExternalInput")
    edgesT_d = nc.dram_tensor("edgesT_s", [NB, NOBJ * NOBJ], F32, kind="ExternalInput")
    outT_d = nc.dram_tensor("outT", [D, ROWS], F32, kind="ExternalOutput")

    from contextlib import ExitStack
    with tile.TileContext(nc) as tc, ExitStack() as ctx:
        consts = ctx.enter_context(tc.tile_pool(name="consts", bufs=1))
        uvp = ctx.enter_context(tc.tile_pool(name="uv", bufs=1))
        big = ctx.enter_context(tc.tile_pool(name="big", bufs=2))
        statp = ctx.enter_context(tc.tile_pool(name="stats", bufs=1))
        psum = ctx.enter_context(tc.tile_pool(name="psum", bufs=4, space="PSUM"))
        dram = ctx.enter_context(tc.tile_pool(name="dram", bufs=1, space="DRAM"))

        # ---------------- setup: load params + state ----------------
        sT = consts.tile([D, ROWS], F32)
        nc.sync.dma_start(out=sT[:], in_=stateT_d.ap())
        w1a = consts.tile([D, F], F32)
        w1b = consts.tile([D, F], F32)
        w2k = consts.tile([D, 2, D], F32)  # [k-half][128,128] tiles of w2T
        fw1 = consts.tile([D, 2, F], F32)  # [128, k-half, 256]
        fw2 = consts.tile([D, 2, D], F32)
        nc.sync.dma_start(out=w1a[:], in_=w1aT_d.ap())
        nc.sync.dma_start(out=w1b[:], in_=w1bT_d.ap())
        if SUB >= 2:
            nc.sync.dma_start(out=w2k[:], in_=w2T_d.ap().rearrange("(k p) d -> p k d", p=D))
            nc.sync.dma_start(out=fw1[:], in_=fw1T_d.ap().rearrange("(k p) f -> p k f", p=D))
            nc.sync.dma_start(out=fw2[:], in_=fw2T_d.ap().rearrange("(k p) d -> p k d", p=D))

        def fvec(dh, nm):  # [256] dram vector -> [128, 2] feature-major sbuf
            t = consts.tile([D, 2], F32, tag=nm, name=nm)
            nc.sync.dma_start(out=t[:], in_=dh.ap().rearrange("(h p) -> p h", p=D))
            return t

        def dvec(dh, nm):  # [128] -> [128, 1]
            t = consts.tile([D, 1], F32, tag=nm, name=nm)
            nc.sync.dma_start(out=t[:], in_=dh.ap().rearrange("(h p) -> p h", p=D))
            return t

        g1c, be1c = fvec(g1_d, "g1c"), fvec(be1_d, "be1c")
        g2c, be2c = fvec(g2_d, "g2c"), fvec(be2_d, "be2c")
        b2c, fb2c = dvec(b2_d, "b2c"), dvec(fb2_d, "fb2c")
        b2x32 = consts.tile([D, 1], F32)
        nc.vector.tensor_scalar_mul(b2x32[:], b2c[:], float(NOBJ))

        if STAGE <= 0:
            nc.sync.dma_start(out=outT_d.ap(), in_=sT[:])
            return nc
        # ---------------- u/v matmuls:  UT/VT[fh] = [128f, 1024(b,*)] --------
        UT = [uvp.tile([D, ROWS], F32, tag=f"UT{h}", name=f"UT{h}") for h in range(2)]
        VT = [uvp.tile([D, ROWS], F32, tag=f"VT{h}", name=f"VT{h}") for h in range(2)]
        for fh in range(2):
            for dst, w in ((UT, w1a), (VT, w1b)):
                for nh in range(2):
                    ps = psum.tile([D, 512], F32, bufs=3)
                    nc.tensor.matmul(ps[:], w[:, fh * D:(fh + 1) * D],
                                     sT[:, nh * 512:(nh + 1) * 512],
                                     start=True, stop=True)
                    nc.scalar.activation(out=dst[fh][:, nh * 512:(nh + 1) * 512],
                                         in_=ps[:], func=AF.Copy)

        # ---------------- sum(H) via degrees:  sum e*(u+v) = deg.u + cdeg.v ---
        esb = consts.tile([NB, NOBJ * NOBJ], F32)
        nc.sync.dma_start(out=esb[:], in_=edges_d.ap())
        deg = statp.tile([NB, NOBJ], F32)    # [b, i] row degree
        nc.vector.reduce_sum(deg[:], esb[:].rearrange("p (i j) -> p i j", j=NOBJ),
                             axis=mybir.AxisListType.X)
        cdeg = statp.tile([NB, NOBJ], F32)   # [b, j] col degree
        nc.vector.reduce_sum(cdeg[:], esb[:].rearrange("p (i j) -> p j i", j=NOBJ),
                             axis=mybir.AxisListType.X)
        degd = dram.tile([NB, NOBJ], F32, tag="degd")
        cdegd = dram.tile([NB, NOBJ], F32, tag="cdegd")
        nc.sync.dma_start(out=degd[:], in_=deg[:])
        nc.sync.dma_start(out=cdegd[:], in_=cdeg[:])
        degrep = statp.tile([D, ROWS], F32)
        nc.sync.dma_start(
            out=degrep[:].rearrange("p (b i) -> p b i", i=NOBJ),
            in_=degd[:].partition_broadcast(D))
        cdegrep = statp.tile([D, ROWS], F32)
        nc.sync.dma_start(
            out=cdegrep[:].rearrange("p (b j) -> p b j", j=NOBJ),
            in_=cdegd[:].partition_broadcast(D))
        shpart = statp.tile([D, 4], F32)     # col = fh*2 + {u,v}
        ttrs = statp.tile([D, ROWS], F32)
        for fh in range(2):
            for uv, (src, rep) in enumerate(((UT[fh], degrep), (VT[fh], cdegrep))):
                nc.vector.tensor_mul(ttrs[:], src[:], rep[:])
                nc.vector.reduce_sum(shpart[:, 2 * fh + uv:2 * fh + uv + 1], ttrs[:],
                                     axis=mybir.AxisListType.X)

        # ---------------- sum(H^2) via algebra ------------------------------
        # sum e*(u+v)^2 = sum_i deg*u^2 + sum_j cdeg*v^2 + 2*sum e*u*v
        # cross term: sum_f' e u v = w1a_f^T M w1b_f,  M = sum_b S_b^T E_b S_b
        sq4 = statp.tile([D, 4], F32)        # col = fh*2 + {u2, v2}
        for fh in range(2):
            for uv, (src_, rep) in enumerate(((UT[fh], degrep), (VT[fh], cdegrep))):
                usq = big.tile([D, ROWS], F32, tag="w", bufs=3, name=f"usq{fh}{uv}")
                nc.scalar.activation(out=usq[:], in_=src_[:], func=AF.Square)
                nc.vector.tensor_mul(ttrs[:], usq[:], rep[:])
                nc.vector.reduce_sum(sq4[:, 2 * fh + uv:2 * fh + uv + 1], ttrs[:],
                                     axis=mybir.AxisListType.X)

        ones_mat = statp.tile([D, D], F32)
        nc.vector.memset(ones_mat[:], 1.0)
        mps = psum.tile([D, D], F32, tag="mps", bufs=1, name="mps")
        for g in range(NG):
            spack = big.tile([D, D], F32, tag="spack", bufs=3, name=f"spack{g}")
            nc.sync.dma_start(out=spack[:], in_=staterm_d.ap()[g * D:(g + 1) * D, :])
            etb = big.tile([D, D], F32, tag="etb", bufs=3, name=f"etb{g}")
            nc.vector.memset(etb[:], 0.0)
            for bs in range(GB):
                nc.sync.dma_start(
                    out=etb[bs * NOBJ:(bs + 1) * NOBJ, bs * NOBJ:(bs + 1) * NOBJ],
                    in_=edgesT_d.ap()[g * GB + bs].rearrange("(j i) -> j i", i=NOBJ))
            esg = psum.tile([D, D], F32, tag="esg", bufs=2, name=f"esg{g}")
            nc.tensor.matmul(esg[:], etb[:], spack[:], start=True, stop=True)
            esgs = big.tile([D, D], F32, tag="esgs", bufs=3, name=f"esgs{g}")
            nc.scalar.activation(out=esgs[:], in_=esg[:], func=AF.Copy)
            nc.tensor.matmul(mps[:], spack[:], esgs[:],
                             start=(g == 0), stop=(g == NG - 1))
        msb = statp.tile([D, D], F32)
        nc.scalar.activation(out=msb[:], in_=mps[:], func=AF.Copy)
        t1p = psum.tile([D, F], F32, tag="t1p", bufs=1, name="t1p")
        nc.tensor.matmul(t1p[:], msb[:], w1a[:], start=True, stop=True)
        t2 = statp.tile([D, F], F32)
        nc.vector.tensor_mul(t2[:], t1p[:], w1b[:])
        crp = psum.tile([D, F], F32, tag="crp", bufs=1, name="crp")
        nc.tensor.matmul(crp[:], ones_mat[:], t2[:], start=True, stop=True)
        crsb = statp.tile([1, F], F32)
        nc.scalar.activation(out=crsb[:], in_=crp[0:1, :], func=AF.Copy)
        crd = dram.tile([1, F], F32, tag="crd", name="crd")
        nc.sync.dma_start(out=crd[:], in_=crsb[:])
        crossc = statp.tile([D, 2], F32)
        nc.sync.dma_start(out=crossc[:],
                          in_=crd[:].rearrange("x (h p) -> (x p) h", p=D))

        # ---------------- BN1 stats allreduce + coeffs ----------------
        stat1 = statp.tile([D, 4], F32)
        nc.vector.reduce_sum(stat1[:, 0:2],
                             shpart[:].rearrange("p (fh uv) -> p fh uv", uv=2),
                             axis=mybir.AxisListType.X)
        sqsum = statp.tile([D, 2], F32)
        nc.vector.reduce_sum(sqsum[:],
                             sq4[:].rearrange("p (fh uv) -> p fh uv", uv=2),
                             axis=mybir.AxisListType.X)
        cr2 = statp.tile([D, 2], F32)
        nc.vector.tensor_scalar_mul(cr2[:], crossc[:], 2.0)
        nc.vector.tensor_add(stat1[:, 2:4], sqsum[:], cr2[:])
        cc1_in = dram.tile([D, 4], F32, tag="cc1i")
        cc1_out = dram.tile([D, 4], F32, tag="cc1o")
        nc.sync.dma_start(out=cc1_in[:], in_=stat1[:])
        if NO_CC:
            nc.sync.dma_start(out=cc1_out[:], in_=cc1_in[:])
        else:
            nc.gpsimd.collective_compute(
                "AllReduce", ALU.add, replica_groups=[list(range(NCORES))],
                ins=[cc1_in[:].opt()], outs=[cc1_out[:].opt()])
        statg1 = statp.tile([D, 4], F32)
        nc.sync.dma_start(out=statg1[:], in_=cc1_out[:])

        if STAGE <= 2:
            nc.sync.dma_start(out=outT_d.ap(), in_=degrep[:])
            return nc
        epsc = statp.tile([D, 1], F32)
        nc.vector.memset(epsc[:], EPS)

        def bn_coeffs(statg, n_rows, gc, bec, pool, pre):
            # statg [128, 4] = [sumH(2fh), sumH2(2fh)] -> a=[128,2], z=[128,2]
            def tl(nm):
                return pool.tile([D, 2], F32, tag=pre + nm, name=pre + nm)
            mean, msq, var = tl("mean"), tl("msq"), tl("var")
            sd, rstd, a, ma, z = tl("sd"), tl("rstd"), tl("a"), tl("ma"), tl("z")
            nc.scalar.activation(out=mean[:], in_=statg[:, 0:2], func=AF.Copy,
                                 scale=1.0 / n_rows)
            nc.scalar.activation(out=msq[:], in_=mean[:], func=AF.Square)
            # var = statg[2:4]/N - mean^2   (one fused scalar_tensor_tensor)
            nc.vector.scalar_tensor_tensor(
                out=var[:], in0=statg[:, 2:4], scalar=1.0 / n_rows,
                in1=msq[:], op0=ALU.mult, op1=ALU.subtract)
            nc.scalar.activation(out=sd[:], in_=var[:], func=AF.Sqrt, bias=epsc[:])
            nc.vector.reciprocal(out=rstd[:], in_=sd[:])
            nc.vector.tensor_mul(a[:], gc[:], rstd[:])
            # z = beta - mean*a
            nc.vector.tensor_mul(ma[:], mean[:], a[:])
            nc.vector.tensor_sub(z[:], bec[:], ma[:])
            return a, z

        a1, z1 = bn_coeffs(statg1, N1 / (NCORES if NO_CC else 1), g1c, be1c, statp, "bn1_")

        # ---------------- fold a1 into u/v: redo matmuls with a1*W1 ----------
        a1d = dram.tile([1, F], F32, tag="a1d", name="a1d")
        nc.sync.dma_start(out=a1d[:].rearrange("x (h p) -> (x p) h", p=D), in_=a1[:])
        a1rep = statp.tile([D, F], F32)
        nc.sync.dma_start(out=a1rep[:],
                          in_=a1d[:].rearrange("x f -> (x f)").partition_broadcast(D))
        w1as = statp.tile([D, F], F32)
        nc.vector.tensor_mul(w1as[:], w1a[:], a1rep[:])
        w1bs = statp.tile([D, F], F32)
        nc.vector.tensor_mul(w1bs[:], w1b[:], a1rep[:])
        UA = [uvp.tile([D, ROWS], F32, tag=f"UT{h}", name=f"UA{h}") for h in range(2)]
        VA = [uvp.tile([D, ROWS], F32, tag=f"VT{h}", name=f"VA{h}") for h in range(2)]
        for fh in range(2):
            for dst, w in ((UA, w1as), (VA, w1bs)):
                for nh in range(2):
                    ps = psum.tile([D, 512], F32, bufs=3)
                    nc.tensor.matmul(ps[:], w[:, fh * D:(fh + 1) * D],
                                     sT[:, nh * 512:(nh + 1) * 512],
                                     start=True, stop=True)
                    nc.scalar.activation(out=dst[fh][:, nh * 512:(nh + 1) * 512],
                                         in_=ps[:], func=AF.Copy)

        # ------- big phase: W = ua+va ; H = W*e ; m = LRelu(H + z0) ; sum_j --
        msumS = [statp.tile([D, ROWS], F32, tag=f"msum{h}", name=f"msum{h}") for h in range(2)]
        pend = None  # (mt, fh, g) whose j-reduce is deferred one tile (DVE FIFO)
        for g in range(NG):
            erep = big.tile([D, CG], F32, tag="erep", bufs=2)
            esl = edges_d.ap()[g * GB:(g + 1) * GB, :]
            nc.gpsimd.dma_start(
                out=erep[:].rearrange("p (b c) -> p b c", b=GB),
                in_=esl.partition_broadcast(D))
            for fh in range(2):
                csl = slice(g * GB * NOBJ, (g + 1) * GB * NOBJ)
                u4 = (UA[fh][:, csl].rearrange("p (b i) -> p b i", i=NOBJ)
                      .unsqueeze(3).broadcast_to([D, GB, NOBJ, NOBJ]))
                v4 = (VA[fh][:, csl].rearrange("p (b j) -> p b j", j=NOBJ)
                      .unsqueeze(2).broadcast_to([D, GB, NOBJ, NOBJ]))
                wt = big.tile([D, CG], F32, tag="w", bufs=3, name=f"wt{g}{fh}")
                nc.vector.tensor_add(
                    wt[:].rearrange("p (b i j) -> p b i j", i=NOBJ, j=NOBJ), u4, v4)
                ht = big.tile([D, CG], F32, tag="h", bufs=3, name=f"ht{g}{fh}")
                nc.gpsimd.tensor_mul(
                    ht[:].rearrange("p (b c) -> p b c", b=GB),
                    wt[:].rearrange("p (b c) -> p b c", b=GB),
                    erep[:].rearrange("p (b c) -> p b c", b=GB))
                mt = big.tile([D, CG], F32, tag="w", bufs=3, name=f"mt{g}{fh}")
                nc.scalar.activation(out=mt[:], in_=ht[:], func=AF.Prelu,
                                     scale=1.0, bias=z1[:, fh:fh + 1], alpha=SLOPE)
                if pend is not None:
                    pmt, pfh, pg = pend
                    nc.vector.reduce_sum(
                        out=msumS[pfh][:, pg * GB * NOBJ:(pg + 1) * GB * NOBJ],
                        in_=pmt[:].rearrange("p (r j) -> p r j", j=NOBJ),
                        axis=mybir.AxisListType.X)
                pend = (mt, fh, g)
        pmt, pfh, pg = pend
        nc.vector.reduce_sum(
            out=msumS[pfh][:, pg * GB * NOBJ:(pg + 1) * GB * NOBJ],
            in_=pmt[:].rearrange("p (r j) -> p r j", j=NOBJ),
            axis=mybir.AxisListType.X)

        if STAGE <= 4:
            nc.sync.dma_start(out=outT_d.ap(), in_=msumS[0][:])
            return nc
        # ---------------- aggT = 32*(W2 @ avg + b2) ----------------
        aggT = big.tile([D, ROWS], F32, tag="h", bufs=3, name="aggT")
        for nh in range(2):
            ps = psum.tile([D, 512], F32, bufs=3)
            nc.tensor.matmul(ps[:], w2k[:, 0, :], msumS[0][:, nh * 512:(nh + 1) * 512],
                             start=True, stop=False)
            nc.tensor.matmul(ps[:], w2k[:, 1, :], msumS[1][:, nh * 512:(nh + 1) * 512],
                             start=False, stop=True)
            nc.scalar.activation(out=aggT[:, nh * 512:(nh + 1) * 512], in_=ps[:],
                                 func=AF.Identity, bias=b2x32[:], scale=1.0)

        # ---------------- layer 2: H2 = FW1 @ [sT; aggT], stats -------------
        H2 = [statp.tile([D, ROWS], F32, tag=f"h2_{h}", name=f"h2_{h}") for h in range(2)]
        st2part = statp.tile([D, 8], F32)  # col = s*4 + fh*2 + nh
        for fh in range(2):
            for nh in range(2):
                ps = psum.tile([D, 512], F32, bufs=3)
                nc.tensor.matmul(ps[:], fw1[:, 0, fh * D:(fh + 1) * D],
                                 sT[:, nh * 512:(nh + 1) * 512], start=True, stop=False)
                nc.tensor.matmul(ps[:], fw1[:, 1, fh * D:(fh + 1) * D],
                                 aggT[:, nh * 512:(nh + 1) * 512], start=False, stop=True)
                c1 = 0 * 4 + fh * 2 + nh
                c2 = 1 * 4 + fh * 2 + nh
                nc.scalar.activation(out=H2[fh][:, nh * 512:(nh + 1) * 512], in_=ps[:],
                                     func=AF.Copy,
                                     accum_out=st2part[:, c1:c1 + 1])
                sq2 = statp.tile([D, 512], F32, tag="sq2")
                nc.scalar.activation(out=sq2[:], in_=H2[fh][:, nh * 512:(nh + 1) * 512],
                                     func=AF.Square,
                                     accum_out=st2part[:, c2:c2 + 1])

        stat2 = statp.tile([D, 4], F32)  # [sumH2(2fh), sumH2sq(2fh)]
        nc.vector.reduce_sum(stat2[:],
                             st2part[:].rearrange("p (s fh nh) -> p (s fh) nh", s=2, nh=2),
                             axis=mybir.AxisListType.X)
        cc2_in = dram.tile([D, 4], F32, tag="cc2i")
        cc2_out = dram.tile([D, 4], F32, tag="cc2o")
        nc.sync.dma_start(out=cc2_in[:], in_=stat2[:])
        if NO_CC:
            nc.sync.dma_start(out=cc2_out[:], in_=cc2_in[:])
        else:
            nc.gpsimd.collective_compute(
                "AllReduce", ALU.add, replica_groups=[list(range(NCORES))],
                ins=[cc2_in[:].opt()], outs=[cc2_out[:].opt()])
        statg2 = statp.tile([D, 4], F32)
        nc.sync.dma_start(out=statg2[:], in_=cc2_out[:])
        a2, z2 = bn_coeffs(statg2, N2 / (NCORES if NO_CC else 1), g2c, be2c, statp, "bn2_")

        # ---------------- m2 = Prelu(a2*H2+z2); outT = FW2 @ m2 + fb2 -------
        m2 = [big.tile([D, ROWS], F32, tag="w", bufs=3, name=f"m2_{h}") for h in range(2)]
        for fh in range(2):
            nc.scalar.activation(out=m2[fh][:], in_=H2[fh][:], func=AF.Prelu,
                                 scale=a2[:, fh:fh + 1], bias=z2[:, fh:fh + 1],
                                 alpha=SLOPE)
        outT = big.tile([D, ROWS], F32, tag="h", bufs=3, name="outT")
        for nh in range(2):
            ps = psum.tile([D, 512], F32, bufs=3)
            nc.tensor.matmul(ps[:], fw2[:, 0, :], m2[0][:, nh * 512:(nh + 1) * 512],
                             start=True, stop=False)
            nc.tensor.matmul(ps[:], fw2[:, 1, :], m2[1][:, nh * 512:(nh + 1) * 512],
                             start=False, stop=True)
            nc.scalar.activation(out=outT[:, nh * 512:(nh + 1) * 512], in_=ps[:],
                                 func=AF.Identity, bias=fb2c[:], scale=1.0)
        nc.sync.dma_start(out=outT_d.ap(), in_=outT[:])
    return nc


def _build_nc_staged():
    nc = _build_nc()
    nc.compile()
    return nc


_NC_CACHE = {}


def _get_nc():
    if "nc" not in _NC_CACHE:
        _NC_CACHE["nc"] = _build_nc_staged()
    return _NC_CACHE["nc"]


def _make_in_maps(inputs):
    return _prep_in_maps(**inputs)


def _prep_in_maps(state, edges, msg_w1, msg_b1, msg_gamma, msg_beta, msg_w2,
                  msg_b2, fin_w1, fin_b1, fin_gamma, fin_beta, fin_w2, fin_b2,
                  **_unused):
    f32 = np.float32
    state = np.ascontiguousarray(np.asarray(state, f32))
    edges = np.ascontiguousarray(np.asarray(edges, f32))

    # replicated params, pre-transposed to device layout (lhsT = K x M)
    w1aT = np.ascontiguousarray(np.asarray(msg_w1, f32)[:, :D].T)    # [128, 256]
    w1bT = np.ascontiguousarray(np.asarray(msg_w1, f32)[:, D:].T)    # [128, 256]
    w2T = np.ascontiguousarray(np.asarray(msg_w2, f32).T)            # [256, 128]
    fw1T = np.ascontiguousarray(np.asarray(fin_w1, f32).T)           # [256, 256]
    fw2T = np.ascontiguousarray(np.asarray(fin_w2, f32).T)           # [256, 128]
    shared = {
        "w1aT": w1aT, "w1bT": w1bT, "w2T": w2T, "fw1T": fw1T, "fw2T": fw2T,
        "g1": np.ascontiguousarray(np.asarray(msg_gamma, f32)),
        "be1": np.ascontiguousarray(np.asarray(msg_beta, f32)),
        "b2": np.ascontiguousarray(np.asarray(msg_b2, f32)),
        "g2": np.ascontiguousarray(np.asarray(fin_gamma, f32)),
        "be2": np.ascontiguousarray(np.asarray(fin_beta, f32)),
        "fb2": np.ascontiguousarray(np.asarray(fin_b2, f32)),
    }
    in_maps = []
    for c in range(NCORES):
        sh = state[c * NB:(c + 1) * NB].reshape(ROWS, D)
        ed = edges[c * NB:(c + 1) * NB]
        in_maps.append({
            "stateT": np.ascontiguousarray(sh.T),
            "state_rm": np.ascontiguousarray(sh),
            "edges_s": np.ascontiguousarray(ed),
            "edgesT_s": np.ascontiguousarray(
                ed.reshape(NB, NOBJ, NOBJ).transpose(0, 2, 1).reshape(NB, -1)),
            **shared,
        })
    return in_maps


def kernel(**inputs):
    in_maps = _prep_in_maps(**inputs)
    nc = _get_nc()
    res = run_bass_kernel_spmd(nc, in_maps, core_ids=list(range(NCORES)))
    out = np.empty((B, NOBJ, D), np.float32)
    for c in range(NCORES):
        outT = res.results[c]["outT"]                       # [128, 1024]
        out[c * NB:(c + 1) * NB] = outT.T.reshape(NB, NOBJ, D)
    return out


if __name__ == "__main__":
    rng = np.random.default_rng(0)
    print("smoke-building nc...")
    _get_nc()
    print("built OK")
